# revision 25
# baseline (speedup 1.0000x reference)
"""MetaConvSmoother Trainium2 kernel (Bass/Tile), data-parallel over 8 NeuronCores.

v4: SBUF-resident pipeline, software-pipelined across samples.

Per core (8 samples):
  - hypernet MLPs (9 -> 100 -> 147, exact gelu) on PE + ACT
  - per-sample conv kernels staged as zero-padded tables in DRAM;
    bands loaded as overlapping windows Bf[p, m] = T[p + m] with
    all-positive strides.  fc2_w2/fc2_b2 arrive ROW-FLIPPED in ky
    (host-side) so every table scatter is an ascending batched DMA;
    kernelA_flip likewise feeds the A tables.
  - parity trick: window band on ASCENDING rhs -> DESCENDING output and
    vice versa.  Chain: Ax (PE-reversed A band, asc->asc) -> r asc ->
    stage1 (window, asc->desc) -> tmp desc -> stage2 (window,
    desc->asc) -> out.  Only the 3-slot A band is PE-reversed.
  - r and tmp live in SBUF: PSUM chunks go through base-0 staging
    tiles (DVE) then SBUF->SBUF DMA scatters into halo-tiled rhs
    tiles (compute engines cannot address partitions off 32-alignment,
    DMA can).
  - out = x + G2 accumulated ON THE PE: const double-identity bands
    (D_full / D_clip) add the x tiles into the stage-2 PSUM, removing
    the x2 reload and its DVE dependency chain.
  - per-iteration PE order: rev(s), Ax(s), stage2(s-1), stage1(s) so
    scatter latencies hide behind ready matmul work.
"""
import numpy as np

import concourse.bass as bass
import concourse.mybir as mybir
from concourse import bacc, bass_utils
from concourse.tile import TileContext

F32 = mybir.dt.float32
F32R = mybir.dt.float32r

S = 8          # samples per core
N = 512
ML = 3
KK = 7
NCORES = 8

TBL = 255                        # elements per slot table
BANDW = 128
NSLOTW = 42                      # 21 S1 + 21 S2
SLOT_S1 = 0
SLOT_S2 = 21
BFW = NSLOTW * BANDW             # 5376
TBLA_TOTAL = S * 3 * TBL         # 6120
TBLW_TOTAL = S * NSLOTW * TBL    # 85680

# Ax chunks: (o0, M); input x rows [o0-1, o0+127)
AX_CH = [(0, 126), (126, 126), (252, 126), (378, 126), (504, 8)]
# stage chunks: (o0, M); 122-row output chunks
S7_CH = [(0, 122), (122, 122), (244, 122), (366, 122), (488, 24)]
# rhs tile row starts (halo 3): tile k holds rows [RS[k], RS[k]+128)
RS = [-3, 119, 241, 363, 485]
# stage2 identity-add pairs: chunk j' -> [(xt tile j, shift, clip)]
ADD_PAIRS = [
    [(0, 1, False)],
    [(0, 123, True), (1, -3, False)],
    [(1, 119, True), (2, -7, False)],
    [(2, 115, True), (3, -11, False)],
    [(3, 111, True), (4, -15, False)],
]


def _sub_ap(base_ap, pattern, offset):
    """Custom access-pattern view: list of [step, count] pairs + elem offset."""
    a = base_ap.copy()
    v = a.ap
    v.clear()
    for p in pattern:
        v.append(list(p))
    a.offset = base_ap.offset + offset
    return a


def _overlaps_asc(o0, M):
    out = []
    for k, rs in enumerate(RS):
        lo = max(o0, rs, 0)
        hi = min(o0 + M, rs + 128, N)
        if hi > lo:
            out.append((k, lo - rs, lo - o0, hi - lo))
    return out


def _overlaps_desc(o0, M, qoff):
    out = []
    for k, rs in enumerate(RS):
        ck = rs + 127
        lo = max(o0, rs, 0)
        hi = min(o0 + M, rs + 128, N)
        if hi > lo:
            p0 = ck - (hi - 1)
            q0 = o0 + 121 - (hi - 1) - qoff
            out.append((k, p0, q0, hi - lo))
    return out


def build_kernel(nc):
    x = nc.dram_tensor("x", [S, N, N], F32, kind="ExternalInput").ap()
    f = nc.dram_tensor("f", [S, N, N], F32, kind="ExternalInput").ap()
    ka = nc.dram_tensor("kernelA", [S, 9], F32, kind="ExternalInput").ap()
    kaf = nc.dram_tensor("kernelA_flip", [S, 9], F32,
                         kind="ExternalInput").ap()
    fc_w1 = [nc.dram_tensor(f"fc{i}_w1", [100, 9], F32, kind="ExternalInput").ap()
             for i in (1, 2)]
    fc_b1 = [nc.dram_tensor(f"fc{i}_b1", [100], F32, kind="ExternalInput").ap()
             for i in (1, 2)]
    fc_w2 = [nc.dram_tensor(f"fc{i}_w2", [147, 100], F32, kind="ExternalInput").ap()
             for i in (1, 2)]
    fc_b2 = [nc.dram_tensor(f"fc{i}_b2", [147], F32, kind="ExternalInput").ap()
             for i in (1, 2)]
    out = nc.dram_tensor("out", [S, N, N], F32, kind="ExternalOutput").ap()

    with TileContext(nc) as tc:
        with (
            tc.tile_pool(name="dram", bufs=1, space="DRAM") as dpool,
            tc.tile_pool(name="const", bufs=1) as cpool,
            tc.tile_pool(name="mlp", bufs=1) as mpool,
            tc.tile_pool(name="bands", bufs=2) as bpool,
            tc.tile_pool(name="banda", bufs=2) as bapool,
            tc.tile_pool(name="rt", bufs=2) as rtpool,
            tc.tile_pool(name="tp", bufs=2) as tppool,
            tc.tile_pool(name="xa", bufs=2) as xa_pool,
            tc.tile_pool(name="fr", bufs=3) as fr_pool,
            tc.tile_pool(name="ftp", bufs=2) as ft_pool,
            tc.tile_pool(name="tm3", bufs=3) as tm3_pool,
            tc.tile_pool(name="psA", bufs=1, space="PSUM") as psA,
            tc.tile_pool(name="ps1", bufs=4, space="PSUM") as ps1,
            tc.tile_pool(name="ps2", bufs=2, space="PSUM") as ps2,
            tc.tile_pool(name="psx", bufs=1, space="PSUM") as psx,
        ):
            tablesA = dpool.tile([TBLA_TOTAL], F32)
            tablesW = dpool.tile([TBLW_TOTAL], F32)

            # ---- zero-fill tables (6120 = 8*765; 85680 = 112*765)
            ztA = tm3_pool.tile([8, 765], F32, name="ztA", tag="tm3")
            nc.vector.memset(ztA, 0.0)
            nc.gpsimd.dma_start(_sub_ap(tablesA, [[765, 8], [1, 765]], 0), ztA)
            ztW = tm3_pool.tile([112, 765], F32, name="ztW", tag="tm3")
            nc.vector.memset(ztW, 0.0)
            nc.gpsimd.dma_start(
                _sub_ap(tablesW, [[765, 112], [1, 765]], 0), ztW)

            # ---- constants (one-time staging shares the tm3 slots;
            # ident is allocated LAST so its slot is recycled latest)
            # anti-diagonal reversal Rev[k,p] = d(k+p=127)
            rev_f = tm3_pool.tile([128, 128], F32, name="rev_f", tag="tm3")
            nc.gpsimd.memset(rev_f, 0.0)
            nc.gpsimd.affine_select(
                out=rev_f, in_=rev_f, compare_op=mybir.AluOpType.not_equal,
                fill=1.0, base=-127, pattern=[[1, 128]], channel_multiplier=1)
            rev = cpool.tile([128, 128], F32R)
            nc.scalar.copy(rev, rev_f)   # round to f32r for the PE
            # double identity D[p, c] = d(p = c - 128), c in [128, 256)
            did_f = tm3_pool.tile([128, 384], F32, name="did_f", tag="tm3")
            nc.gpsimd.memset(did_f, 0.0)
            nc.gpsimd.affine_select(
                out=did_f, in_=did_f, compare_op=mybir.AluOpType.not_equal,
                fill=1.0, base=128, pattern=[[-1, 384]], channel_multiplier=1)
            d_full = cpool.tile([128, 384], F32R)
            nc.scalar.copy(d_full, did_f)
            d_clip = cpool.tile([128, 384], F32R)
            nc.scalar.copy(d_clip, did_f)
            # zero partitions 126,127 of the diagonal = zero cols 254,255
            nc.vector.memset(d_clip.bitcast(F32)[:, 254:256], 0.0)
            ident = tm3_pool.tile([128, 128], F32, name="ident", tag="tm3")
            nc.gpsimd.memset(ident, 0.0)
            nc.gpsimd.affine_select(
                out=ident, in_=ident, compare_op=mybir.AluOpType.not_equal,
                fill=1.0, base=0, pattern=[[-1, 128]], channel_multiplier=1)

            # ---- A tables from host-flipped kernelA (no MLP dependency):
            # T_A[(s*3+kx)*255 + 125 + ky'] = A_flip[ky', kx]
            vTf = mpool.tile([9, S], F32, name="vTf")
            nc.sync.dma_start(vTf, kaf.rearrange("s k -> k s"))
            for kyf in range(3):
                nc.gpsimd.dma_start(
                    _sub_ap(tablesA, [[TBL, 3], [3 * TBL, S]], 125 + kyf),
                    vTf[3 * kyf:3 * kyf + 3, :])

            # ---------------- MLP + weight staging ----------------
            vT = mpool.tile([9, S], F32)
            nc.sync.dma_start(vT, ka.rearrange("s k -> k s"))

            w_sb = {}  # (layer i, map m) -> [49, S] conv weights
            for i in range(2):
                w1n = mpool.tile([100, 9], F32, name=f"w1n{i}")
                nc.sync.dma_start(w1n, fc_w1[i])
                W1T = mpool.tile([9, 100], F32, name=f"W1T{i}")
                t1 = psx.tile([9, 100], F32, name=f"t1_{i}", tag="aux")
                nc.tensor.transpose(t1, w1n, ident[:100, :100])
                nc.vector.tensor_copy(W1T, t1)

                b1 = mpool.tile([100, 1], F32, name=f"b1_{i}")
                nc.sync.dma_start(b1, fc_b1[i].unsqueeze(1))

                w2n_a = mpool.tile([128, 100], F32, name=f"w2na{i}")
                nc.sync.dma_start(w2n_a, fc_w2[i][0:128, :])
                w2n_b = mpool.tile([19, 100], F32, name=f"w2nb{i}")
                nc.sync.dma_start(w2n_b, fc_w2[i][128:147, :])
                W2T = mpool.tile([100, 147], F32, name=f"W2T{i}")
                tr_a = psx.tile([100, 128], F32, name=f"tra{i}", tag="aux")
                nc.tensor.transpose(tr_a, w2n_a, ident)
                nc.vector.tensor_copy(W2T[:, 0:128], tr_a)
                tr_b = psx.tile([100, 19], F32, name=f"trb{i}", tag="aux")
                nc.tensor.transpose(tr_b, w2n_b, ident[:19, :19])
                nc.vector.tensor_copy(W2T[:, 128:147], tr_b)

                h_pre = psx.tile([100, S], F32, name=f"hpre{i}", tag="aux")
                nc.tensor.matmul(h_pre, W1T, vT, start=True, stop=True)
                h = mpool.tile([100, S], F32, name=f"h{i}")
                nc.scalar.activation(
                    h, h_pre, mybir.ActivationFunctionType.Gelu, bias=b1)

                for m in range(ML):
                    b2m = mpool.tile([49, 1], F32, name=f"b2_{i}_{m}")
                    nc.sync.dma_start(
                        b2m, fc_b2[i][49 * m:49 * m + 49].unsqueeze(1))
                    wp = psx.tile([49, S], F32, name=f"wp{i}{m}", tag="aux")
                    nc.tensor.matmul(wp, W2T[:, 49 * m:49 * m + 49], h,
                                     start=True, stop=True)
                    wsb = mpool.tile([49, S], F32, name=f"w_{i}_{m}")
                    nc.scalar.activation(
                        wsb, wp, mybir.ActivationFunctionType.Identity,
                        bias=b2m)
                    w_sb[(i, m)] = wsb

            # ---- batched scatters (ascending everywhere; fc2 rows are
            # host-flipped so stage-2 uses the same +ky layout):
            # T_W[(s*42 + base + m*7 + kx)*255 + 121 + ky] = w[ky, kx]
            for i, base in ((0, SLOT_S1), (1, SLOT_S2)):
                for m in range(ML):
                    for ky in range(KK):
                        nc.gpsimd.dma_start(
                            _sub_ap(tablesW,
                                    [[TBL, KK], [NSLOTW * TBL, S]],
                                    (base + m * KK) * TBL + 121 + ky),
                            w_sb[(i, m)][KK * ky:KK * ky + KK, :])

            # ---------------- main per-sample loop (software-pipelined) ----
            bb_t, ba_t, rt_t, tp_t, xt_t = {}, {}, {}, {}, {}

            def band(s, slot, m0, m1):
                b = bb_t[s]
                return b[:, slot * BANDW + m0:slot * BANDW + m1]

            def emit_band_load(s):
                baw = bapool.tile([128, 3 * BANDW], F32R, name=f"baw{s}",
                                  tag="baw")
                nc.sync.dma_start(
                    _sub_ap(baw, [[3 * BANDW, 128], [BANDW, 3], [1, BANDW]],
                            0),
                    _sub_ap(tablesA.bitcast(F32R),
                            [[1, 128], [TBL, 3], [1, BANDW]], s * 3 * TBL))
                ba_t[s] = [baw]      # [0]=window, rev appends reversed
                bb = bpool.tile([128, BFW], F32R, name=f"bb{s}", tag="bb")
                bb_t[s] = bb
                nc.sync.dma_start(
                    _sub_ap(bb, [[BFW, 128], [BANDW, 21], [1, BANDW]], 0),
                    _sub_ap(tablesW.bitcast(F32R),
                            [[1, 128], [TBL, 21], [1, BANDW]],
                            s * NSLOTW * TBL))
                nc.scalar.dma_start(
                    _sub_ap(bb, [[BFW, 128], [BANDW, 21], [1, BANDW]],
                            21 * BANDW),
                    _sub_ap(tablesW.bitcast(F32R),
                            [[1, 128], [TBL, 21], [1, BANDW]],
                            (s * NSLOTW + 21) * TBL))

            def emit_x_load(s):
                # one [128, 5*514] tile; block j holds x rows
                # [126j-1, 126j+127) at col offset 1 (pads at cols 0/513).
                XW = N + 2
                xt = xa_pool.tile([128, 5 * XW], F32R, name=f"xt{s}",
                                  tag="xa")
                if s < 2:
                    # block 4: ones pad (rows >= 512), then col pads
                    nc.gpsimd.memset(
                        xt.bitcast(F32)[:, 4 * XW:5 * XW], 1.0)
                    nc.gpsimd.memset(xt.bitcast(F32)[0:1, 0:XW], 0.0)
                    for j in range(4):
                        nc.gpsimd.memset(
                            xt.bitcast(F32)[:, j * XW:j * XW + 1], 0.0)
                        nc.gpsimd.memset(
                            xt.bitcast(F32)[:, j * XW + N + 1:
                                            j * XW + N + 2], 1.0)
                    # block 4 col 0: zero only the 9 real rows (pad rows
                    # >= 512 keep the 1.0 fill, matching the reference)
                    nc.gpsimd.memset(
                        xt.bitcast(F32)[0:9, 4 * XW:4 * XW + 1], 0.0)
                # block 0: rows 0..126 -> partitions 1..127
                nc.sync.dma_start(
                    _sub_ap(xt, [[5 * XW, 127], [1, N]], 5 * XW + 1),
                    x.bitcast(F32R)[s, 0:127, :])
                # blocks 1..3: rows 126j-1 .. 126j+126
                nc.sync.dma_start(
                    _sub_ap(xt, [[5 * XW, 128], [XW, 3], [1, N]], XW + 1),
                    _sub_ap(x.bitcast(F32R),
                            [[N, 128], [126 * N, 3], [1, N]],
                            (s * N + 125) * N))
                # block 4: rows 503..511 -> partitions 0..8
                nc.sync.dma_start(
                    _sub_ap(xt, [[5 * XW, 9], [1, N]], 4 * XW + 1),
                    x.bitcast(F32R)[s, 503:512, :])
                xt_t[s] = xt

            def emit_rev(s):
                baw = ba_t[s][0]
                ba = bapool.tile([128, 3 * BANDW], F32R, name=f"ba{s}",
                                 tag="ba")
                pr = psx.tile([128, 3 * BANDW], F32, name=f"pr{s}", tag="aux")
                nc.tensor.matmul(pr, rev, baw, start=True, stop=True)
                nc.scalar.copy(ba, pr)
                ba_t[s].append(ba)

            def emit_rhs_tiles(s):
                rt, tp = [], []
                for k in range(5):
                    t = rtpool.tile([128, N + 6], F32R, name=f"rt{s}_{k}",
                                    tag=f"rt{k}")
                    rt.append(t)
                    if s < 2:
                        nc.gpsimd.memset(t.bitcast(F32), 0.0)
                    t2 = tppool.tile([128, ML * (N + 6)], F32R,
                                     name=f"tp{s}_{k}", tag=f"tp{k}")
                    tp.append(t2)
                    if s < 2:
                        nc.gpsimd.memset(t2.bitcast(F32), 0.0)
                rt_t[s], tp_t[s] = rt, tp

            def emit_ax(s):
                ba, rt, xt = ba_t[s][1], rt_t[s], xt_t[s]
                XW = N + 2
                ft = ft_pool.tile([126, 5 * N], F32, name=f"ft{s}", tag="f")
                nc.sync.dma_start(
                    _sub_ap(ft, [[5 * N, 126], [N, 4], [1, N]], 0),
                    _sub_ap(f, [[N, 126], [126 * N, 4], [1, N]],
                            s * N * N))
                nc.sync.dma_start(
                    _sub_ap(ft, [[5 * N, 8], [1, N]], 4 * N),
                    f[s, 504:512, :])
                for j, (o0, M) in enumerate(AX_CH):
                    ps = psA.tile([M, N], F32, name=f"psA{s}_{j}", tag="ax")
                    for kx in range(3):
                        nc.tensor.matmul(
                            ps, ba[:, kx * BANDW:kx * BANDW + M],
                            xt[:, j * XW + kx:j * XW + kx + N],
                            start=(kx == 0), stop=(kx == 2))
                    rf = fr_pool.tile([126, N], F32, name=f"rf{s}_{j}",
                                      tag="rf")
                    nc.vector.tensor_sub(rf[:M, :], ft[:M, j * N:(j + 1) * N],
                                         ps[:M, :])
                    for (k, p0, q0, n) in _overlaps_asc(o0, M):
                        nc.gpsimd.dma_start(
                            _sub_ap(rt[k], [[N + 6, n], [1, N]],
                                    p0 * (N + 6) + 3),
                            _sub_ap(rf.bitcast(F32R), [[N, n], [1, N]],
                                    q0 * N))

            def emit_stage1(s):
                rt, tp = rt_t[s], tp_t[s]
                for j, (o0, M) in enumerate(S7_CH):
                    qoff = 98 if M < 122 else 0   # lhsT col slice for c4
                    tm3 = tm3_pool.tile([122, ML * N], F32,
                                        name=f"tm3_{s}_{j}", tag="tm3")
                    for m in range(ML):
                        ps_ = ps1.tile([122, N], F32, name=f"ps1_{s}_{j}_{m}",
                                       tag="s1")
                        for kx in range(KK):
                            nc.tensor.matmul(
                                ps_[:M, :],
                                band(s, SLOT_S1 + m * KK + kx, qoff,
                                     qoff + M),
                                rt[j][:, kx:kx + N],
                                start=(kx == 0), stop=(kx == KK - 1))
                        nc.vector.tensor_copy(
                            tm3[:M, m * N:(m + 1) * N], ps_[:M, :])
                    for (k, p0, q0, n) in _overlaps_desc(o0, M, qoff):
                        eng = (nc.sync if j % 2 == 0 else
                               nc.scalar) if n > 32 else nc.gpsimd
                        eng.dma_start(
                            _sub_ap(tp[k],
                                    [[ML * (N + 6), n], [N + 6, ML], [1, N]],
                                    p0 * ML * (N + 6) + 3),
                            _sub_ap(tm3.bitcast(F32R),
                                    [[ML * N, n], [N, ML], [1, N]],
                                    q0 * ML * N))

            def emit_stage2(s):
                tp, xt = tp_t[s], xt_t[s]
                XW = N + 2
                ob = ft_pool.tile([122, 5 * N], F32, name=f"ob{s}", tag="f")
                for j, (o0, M) in enumerate(S7_CH):
                    pg = ps2.tile([122, N], F32, name=f"ps2_{s}_{j}", tag="s2")
                    nmm = 21 + len(ADD_PAIRS[j])
                    idx = 0
                    for m in range(ML):
                        for kx in range(KK):
                            nc.tensor.matmul(
                                pg[:M, :],
                                band(s, SLOT_S2 + m * KK + kx, 0, M),
                                tp[j][:, m * (N + 6) + kx:
                                      m * (N + 6) + kx + N],
                                start=(idx == 0), stop=(idx == nmm - 1))
                            idx += 1
                    # out = x + G2 on the PE via double-identity bands
                    for (jx, shift, clip) in ADD_PAIRS[j]:
                        dd = d_clip if clip else d_full
                        nc.tensor.matmul(
                            pg[:M, :], dd[:, 128 + shift:128 + shift + M],
                            xt[:, jx * XW + 1:jx * XW + 1 + N],
                            start=False, stop=(idx == nmm - 1))
                        idx += 1
                    nc.scalar.copy(ob[:M, j * N:(j + 1) * N], pg[:M, :])
                nc.scalar.dma_start(
                    _sub_ap(out, [[N, 122], [122 * N, 4], [1, N]],
                            s * N * N),
                    _sub_ap(ob, [[5 * N, 122], [N, 4], [1, N]], 0))
                nc.scalar.dma_start(
                    out[s, 488:512, :],
                    _sub_ap(ob, [[5 * N, 24], [1, N]], 4 * N))
                del bb_t[s], ba_t[s], rt_t[s], tp_t[s], xt_t[s]

            # prologue
            emit_band_load(0)
            emit_x_load(0)
            for s in range(S):
                emit_rev(s)
                emit_rhs_tiles(s)
                emit_ax(s)
                if s >= 1:
                    emit_stage2(s - 1)
                # prefetch next iteration inputs AFTER stage2(s-1): the
                # buffer-rotation WARs on xt/bb are then already satisfied,
                # so the sync sequencer never stalls on compute progress.
                if s + 1 < S:
                    emit_band_load(s + 1)
                    emit_x_load(s + 1)
                emit_stage1(s)
            emit_stage2(S - 1)
    return nc


_CACHED = None


def _get_nc():
    global _CACHED
    if _CACHED is None:
        nc = bacc.Bacc("TRN2", debug=False, enable_asserts=False,
                       num_devices=NCORES)
        build_kernel(nc)
        nc.compile()
        _CACHED = nc
    return _CACHED


def make_in_maps(x, f, kernelA, fc1_w1, fc1_b1, fc1_w2, fc1_b2,
                 fc2_w1, fc2_b1, fc2_w2, fc2_b2):
    # stage-2 ky flip lives host-side: reorder fc2_w2/fc2_b2 output rows
    # (147 = 3 maps x 7 ky x 7 kx) so the on-device scatter is ascending.
    w2f = np.ascontiguousarray(
        np.asarray(fc2_w2, np.float32).reshape(ML, KK, KK, 100)[:, ::-1]
        .reshape(ML * KK * KK, 100))
    b2f = np.ascontiguousarray(
        np.asarray(fc2_b2, np.float32).reshape(ML, KK, KK)[:, ::-1]
        .reshape(ML * KK * KK))
    shared = {
        "fc1_w1": np.ascontiguousarray(fc1_w1, np.float32),
        "fc1_b1": np.ascontiguousarray(fc1_b1, np.float32),
        "fc1_w2": np.ascontiguousarray(fc1_w2, np.float32),
        "fc1_b2": np.ascontiguousarray(fc1_b2, np.float32),
        "fc2_w1": np.ascontiguousarray(fc2_w1, np.float32),
        "fc2_b1": np.ascontiguousarray(fc2_b1, np.float32),
        "fc2_w2": w2f,
        "fc2_b2": b2f,
    }
    in_maps = []
    for c in range(NCORES):
        sl = slice(S * c, S * (c + 1))
        kac = np.ascontiguousarray(
            kernelA[sl, 0].reshape(S, 9), np.float32)
        kaflip = np.ascontiguousarray(
            kac.reshape(S, 3, 3)[:, ::-1].reshape(S, 9))
        in_maps.append({
            "x": np.ascontiguousarray(x[sl, 0], np.float32),
            "f": np.ascontiguousarray(f[sl, 0], np.float32),
            "kernelA": kac,
            "kernelA_flip": kaflip,
            **shared,
        })
    return in_maps


def kernel(x, f, kernelA, fc1_w1, fc1_b1, fc1_w2, fc1_b2,
           fc2_w1, fc2_b1, fc2_w2, fc2_b2):
    x = np.asarray(x)
    nc = _get_nc()
    in_maps = make_in_maps(x, f, kernelA, fc1_w1, fc1_b1, fc1_w2, fc1_b2,
                           fc2_w1, fc2_b1, fc2_w2, fc2_b2)
    res = bass_utils.run_bass_kernel_spmd(
        nc, in_maps, core_ids=list(range(NCORES)))
    outs = [res.results[c]["out"] for c in range(NCORES)]
    full = np.concatenate(outs, axis=0).reshape(64, 1, N, N).astype(np.float32)
    return full
